# revision 1
# baseline (speedup 1.0000x reference)
"""BitLinear (BitNet-style) kernel for 8 Trainium2 NeuronCores.

Computes: out = input @ (sign(W) * mean(|W|)).T + bias
  input [8192, 2048] f32, W [8192, 2048] f32, bias [8192] f32 -> out [8192, 8192] f32

Sharding: column-parallel over out_features. Core j owns W rows
[j*1024, (j+1)*1024). Each core computes sign() on its shard (scalar
engine) and a local |W| partial sum (vector engine reduce with absolute
value); partial sums are AllReduce'd across the 8 cores so the scale is
the global abs-mean. The GEMM runs in bf16 (sign(W) is exactly
representable; input/weights are rounded host-side), accumulating in
fp32 PSUM. scale (fp32) and bias (fp32) are fused into the PSUM->SBUF
eviction: out = psum * scale + bias.

Layout: host ships input already transposed (inT = input.T, bf16) and
the weight shard transposed (wT = W.T shard, bf16) so both GEMM operands
are K-major as the tensor engine requires; each core writes its out.T
shard [1024, 8192] contiguously and the host re-transposes once.

Perf notes (cost-model + real-HW repeat-slope profiled):
- 2048 matmuls of [K=128]x[M=128 o]x[N=512 t] stream at ~214 ns each —
  the bf16 1-col/cycle floor (~438 us busy); projection ~450 us/core.
- Stationary sign-weights are fp8e4 (+-1 exact): on real HW this removed
  ~90 us/iter of exposed LDWEIGHTS time vs a bf16 stationary (measured
  539 -> 446 us/iter via R-repeat wall-clock slope), since every matmul
  carries its own weight load and bf16 FWL loads don't fully hide.
- Output stores issue on the ACT HWDGE ring so they can't head-of-line
  block the next span's input loads on the SP ring (strict per-ring FIFO).
- The scale chain never touches the in-order PE queue (cross-partition
  sum via DRAM bounce + DVE, broadcast via step-0 DMA), and its small
  DMAs stay off the SP HWDGE FIFO so they can't head-of-line block the
  input loads while waiting on the collective.
- Ramped token spans (512,512,1024,2048x3): early spans use 1 PSUM bank
  per o-group (up to 8 in flight) and a copy-only eviction with the
  scale/bias folded in a second DVE pass, so nothing stalls on the
  AllReduce latency.
"""

import sys

for _p in ("/opt/trn_rl_repo",):
    if _p not in sys.path:
        sys.path.append(_p)

import ml_dtypes
import numpy as np

TOKENS = 8192
D_IN = 2048
D_OUT = 8192
NCORES = 8
OSH = D_OUT // NCORES  # 1024 out features per core
P = 128
KT = D_IN // P         # 16 k-tiles of 128
TQ = 2048              # resident token span
OT = OSH // P          # 8 o-tiles per core
SPAN_SCHEDULE = (512, 512, 1024, 2048, 2048, 2048)

_NC_CACHE = {}


def _build_nc(use_collective=True, repeat=1, dedup_ldw=True):
    import concourse.mybir as mybir
    import concourse.tile as tile
    from concourse import bacc

    f32 = mybir.dt.float32
    bf16 = mybir.dt.bfloat16
    fp8 = mybir.dt.float8e4
    AF = mybir.ActivationFunctionType

    nc = bacc.Bacc("TRN2", target_bir_lowering=False, debug=False,
                   num_devices=NCORES)

    inT = nc.dram_tensor("inT", [D_IN, TOKENS], bf16, kind="ExternalInput")
    wT = nc.dram_tensor("wT", [D_IN, OSH], bf16, kind="ExternalInput")
    bias2d = nc.dram_tensor("bias2d", [P, OT], f32, kind="ExternalInput")
    outT = nc.dram_tensor("outT", [OSH, TOKENS], f32, kind="ExternalOutput")
    cc_in = nc.dram_tensor("cc_in", [1, 8], f32)
    cc_out = nc.dram_tensor("cc_out", [1, 8], f32, addr_space="Shared")
    colsum_dram = nc.dram_tensor("colsum_dram", [P], f32)

    inT_r = inT.ap().rearrange("(k p) t -> p k t", p=P)
    wT_r = wT.ap().rearrange("(k p) o -> p k o", p=P)
    outT_r = outT.ap().rearrange("(o p) t -> p o t", p=P)

    WG = 2 if KT % 2 == 0 else 1   # k-tiles per Sign-activation slice
    # W DMA schedule: small first load so the first stationary tile (and the
    # first matmul) is ready a few us in; bigger loads amortize DMA overhead.
    if KT == 16:
        WSCHED = (2, 2, 4, 4, 4)
    else:
        WSCHED = (KT,)
    NWQ = len(WSCHED)
    WQMAX = max(WSCHED)

    with tile.TileContext(nc) as tc:
        with (
            tc.tile_pool(name="const", bufs=1) as const,
            tc.tile_pool(name="wpool", bufs=1) as wpool,
            tc.tile_pool(name="wstream", bufs=2) as wstream,
            tc.tile_pool(name="small", bufs=1) as small,
            tc.tile_pool(name="inpool", bufs=28) as inpool,
            tc.tile_pool(name="outpool", bufs=2) as outpool,
            tc.tile_pool(name="pmm", bufs=8, space="PSUM") as pmm,
        ):
            bias_sb = const.tile([P, OT], f32)
            nc.gpsimd.dma_start(bias_sb[:], bias2d.ap())

            # PE clock warmup: the HAM gate holds the array at 1.2 GHz until
            # ~3.4us of sustained activity. Burn that window on throwaway
            # matmuls over a zeroed tile while the first weights stream in,
            # so the real matmuls start at 2.4 GHz.
            warm_src = const.tile([P, 256], bf16)
            nc.vector.memset(warm_src[:], 0.0)
            warm_ps = pmm.tile([P, 512], f32, tag="mm", name="warm_ps")
            NWARM = 14
            for wmm in range(NWARM):
                nc.tensor.matmul(warm_ps[0:16, 0:256], warm_src[:, 0:16],
                                 warm_src[:],
                                 start=(wmm == 0), stop=(wmm == NWARM - 1))

            # --- weight shard: sign -> bf16, |W| partial sums ---
            # Sign on ACT; |.| row-sums on DVE (reduce with absolute value);
            # no PE involvement anywhere in the scale chain so the in-order
            # PE queue is never blocked on it.
            sT = wpool.tile([P, KT, OSH], fp8)
            absacc = wpool.tile([P, NWQ], f32)
            k0 = 0
            for g, wq in enumerate(WSCHED):
                wt = wstream.tile([P, WQMAX, OSH], bf16, tag="wt",
                                  name=f"wt{g}")
                nc.sync.dma_start(
                    wt[:, :wq, :], wT_r[:, k0:k0 + wq, :]
                )
                for s in range(0, wq, WG):
                    sl = min(WG, wq - s)
                    nc.scalar.activation(sT[:, k0 + s:k0 + s + sl, :],
                                         wt[:, s:s + sl, :], AF.Sign)
                nc.vector.tensor_reduce(absacc[:, g:g + 1], wt[:, :wq, :],
                                        axis=mybir.AxisListType.XY,
                                        op=mybir.AluOpType.add,
                                        apply_absolute_value=True)
                k0 += wq

            # --- global scale via AllReduce of the scalar partial ---
            colsum = small.tile([P, 1], f32)
            nc.vector.reduce_sum(colsum[:], absacc[:], axis=mybir.AxisListType.X)
            # cross-partition gather via a DRAM bounce (partition axis can't
            # fold into an SBUF free axis) + free-axis reduce
            nc.gpsimd.dma_start(colsum_dram.ap(), colsum[:, 0])
            rowt = small.tile([1, P], f32)
            nc.gpsimd.dma_start(rowt[0:1, :], colsum_dram.ap()[None, :])
            part = small.tile([1, 8], f32)
            nc.vector.memset(part[:], 0.0)
            nc.vector.reduce_sum(part[0:1, 0:1], rowt[0:1, :],
                                 axis=mybir.AxisListType.X)
            # keep the scale chain's DMAs off the SP HWDGE ring: tot8 waits
            # on the collective, and the SP ring is FIFO — it would
            # head-of-line block every subsequent input load.
            nc.gpsimd.dma_start(cc_in.ap(), part[:])
            if use_collective:
                nc.gpsimd.collective_compute(
                    "AllReduce",
                    mybir.AluOpType.add,
                    replica_groups=[list(range(NCORES))],
                    ins=[cc_in.ap()],
                    outs=[cc_out.ap()],
                )
                cc_result = cc_out
            else:
                # timing-model variant (TimelineSim can't model collectives):
                # local partial stands in for the global sum
                nc.gpsimd.dma_start(cc_out.ap(), cc_in.ap())
                cc_result = cc_out
            # broadcast the reduced scalar to all 128 partitions straight
            # from DRAM (step-0 source AP)
            scale_raw = small.tile([P, 1], f32)
            with nc.allow_non_contiguous_dma(reason="scale broadcast"):
                nc.gpsimd.dma_start(scale_raw[:, 0:1],
                                    cc_result.ap()[0:1, 0:1].to_broadcast((P, 1)))
            scale_b = small.tile([P, 1], f32)
            nc.scalar.activation(scale_b[:], scale_raw[:], AF.Copy,
                                 scale=1.0 / float(D_OUT * D_IN))

            # --- main GEMM: outT[o, t] = sum_k sT[k, o] * inT[k, t] ---
            # ramped token spans: tiny first spans use 1 PSUM bank per
            # o-group so up to 7 o-groups accumulate k-incrementally while
            # the first weights/inputs are still arriving from HBM.
            spans = []
            t0 = 0
            for tq in SPAN_SCHEDULE:
                spans.append((t0, tq))
                t0 += tq
            assert t0 == TOKENS
            # repeat>1 re-runs the whole GEMM (same outputs rewritten) so a
            # wall-clock slope over R cancels fixed launch/proxy overheads.
            spans = [(q + r * len(spans), t0, tq)
                     for r in range(repeat)
                     for q, (t0, tq) in enumerate(spans)]
            nspans0 = len(SPAN_SCHEDULE)
            for q, t0, tq in spans:
                ncht = tq // 512
                inq = []
                for k in range(KT):
                    it = inpool.tile([P, TQ], bf16, tag="in",
                                     name=f"in_q{q}_k{k}")
                    nc.sync.dma_start(it[:, :tq], inT_r[:, k, t0:t0 + tq])
                    inq.append(it)
                for o in range(OT):
                    psums = [
                        pmm.tile([P, 512], f32, tag="mm", name=f"pp{q}_{o}_{c}")
                        for c in range(ncht)
                    ]
                    for k in range(KT):
                        lhsT = sT[:, k, o * P:(o + 1) * P]
                        for c in range(ncht):
                            nc.tensor.matmul(
                                psums[c][:], lhsT,
                                inq[k][:, c * 512:(c + 1) * 512],
                                start=(k == 0), stop=(k == KT - 1),
                            )
                    stage = outpool.tile([P, tq], f32, tag=f"stage{tq}",
                                         bufs=(8 if tq <= 512 else 2),
                                         name=f"st{q}_{o}")
                    if q % nspans0 < 3 and q < nspans0:
                        # early spans: scale may still be in flight (the
                        # AllReduce) — evict with a plain copy so the PSUM
                        # bank frees immediately, fold scale+bias in a
                        # second DVE pass before the store.
                        for c in range(ncht):
                            nc.scalar.activation(
                                stage[:, c * 512:(c + 1) * 512], psums[c][:],
                                AF.Copy)
                        nc.vector.tensor_scalar(
                            stage[:], stage[:],
                            scale_b[:, 0:1], bias_sb[:, o:o + 1],
                            mybir.AluOpType.mult, mybir.AluOpType.add)
                    elif q == len(spans) - 1 and o == OT - 1:
                        # very last tile: store per chunk so the final DMA
                        # isn't serialized behind all four evictions
                        for c in range(ncht):
                            nc.scalar.activation(
                                stage[:, c * 512:(c + 1) * 512], psums[c][:],
                                AF.Identity,
                                bias=bias_sb[:, o:o + 1], scale=scale_b[:, 0:1],
                            )
                            eng = nc.scalar if c % 2 == 0 else nc.sync
                            eng.dma_start(
                                outT_r[:, o, t0 + c * 512:t0 + (c + 1) * 512],
                                stage[:, c * 512:(c + 1) * 512])
                        continue
                    else:
                        for c in range(ncht):
                            nc.scalar.activation(
                                stage[:, c * 512:(c + 1) * 512], psums[c][:],
                                AF.Identity,
                                bias=bias_sb[:, o:o + 1], scale=scale_b[:, 0:1],
                            )
                    nc.scalar.dma_start(outT_r[:, o, t0:t0 + tq],
                                      stage[:])

    if dedup_ldw:
        _dedup_ldweights(nc, mybir)
    nc.compile()
    return nc


def _dedup_ldweights(nc, mybir):
    """Drop consecutive InstLdweights that reload the exact same stationary
    AP with only matmuls in between. Tile emits one weight load per matmul
    even when ncht matmuls share a stationary; on HW the redundant loads are
    partially exposed. The following non-self-loading matmuls keep using the
    already-loaded array state. Only waitless/updateless loads are removed."""
    removed = 0
    for bb in nc.m.functions[0].blocks:
        il = bb.instructions
        kept = []
        prev_sig = None
        for i in il:
            if isinstance(i, mybir.InstLdweights):
                sig = str(i.ins[0])
                if (sig == prev_sig and not i.has_wait()
                        and not i.has_update()):
                    nc.inst_map.pop(i.name, None)
                    removed += 1
                    continue
                prev_sig = sig
            elif isinstance(i, mybir.InstMatmult):
                pass
            elif getattr(i, "engine", None) == mybir.EngineType.PE:
                prev_sig = None
            kept.append(i)
        il[:] = kept


def _get_nc():
    if "nc" not in _NC_CACHE:
        _NC_CACHE["nc"] = _build_nc()
    return _NC_CACHE["nc"]


def _make_in_maps(input, weight, bias):
    inT = np.ascontiguousarray(input.T).astype(ml_dtypes.bfloat16)
    wT_full = weight.T  # [D_IN, D_OUT] view
    in_maps = []
    for j in range(NCORES):
        bsh = bias[j * OSH:(j + 1) * OSH]
        in_maps.append({
            "inT": inT,
            "wT": np.ascontiguousarray(
                wT_full[:, j * OSH:(j + 1) * OSH]).astype(ml_dtypes.bfloat16),
            "bias2d": np.ascontiguousarray(
                bsh.reshape(OT, P).T, dtype=np.float32),
        })
    return in_maps


def run(input, weight, bias, trace=False, **spmd_kwargs):
    from concourse.bass_utils import run_bass_kernel_spmd

    nc = _get_nc()
    in_maps = _make_in_maps(np.asarray(input, dtype=np.float32),
                            np.asarray(weight, dtype=np.float32),
                            np.asarray(bias, dtype=np.float32))
    res = run_bass_kernel_spmd(nc, in_maps, core_ids=list(range(NCORES)),
                               trace=trace, **spmd_kwargs)
    outT = np.concatenate([r["outT"] for r in res.results], axis=0)
    out = np.ascontiguousarray(outT.T)
    return out, res


def kernel(input, weight, bias):
    out, _ = run(input, weight, bias, trace=False)
    return out



# revision 5
# speedup vs baseline: 1.7798x; 1.7798x over previous
"""BitLinear (BitNet-style) kernel for 8 Trainium2 NeuronCores.

Computes: out = input @ (sign(W) * mean(|W|)).T + bias
  input [8192, 2048] f32, W [8192, 2048] f32, bias [8192] f32 -> out [8192, 8192] f32

Sharding: column-parallel over out_features. Core j owns W rows
[j*1024, (j+1)*1024). Each core computes sign() on its shard (scalar
engine) and a local |W| partial sum (vector engine reduce with absolute
value); partial sums are AllReduce'd across the 8 cores so the scale is
the global abs-mean.

GEMM precision/speed: the PE's fp8 DoubleRow mode packs two k-planes per
matmul (stationary [128,2,M], moving [128,2,N]) and streams at 0.5
cycles per output column - 2x the bf16 column rate with twice the K per
step (4x MACs/cycle). sign(W) is exactly representable in fp8e4, and the
input is fed as an exact-ish hi+lo pair: x_hi = fp8(x),
x_lo = fp8(x - x_hi), both multiplied against the same sign stationary
into the same PSUM accumulation, recovering ~11 mantissa bits
(measured end-to-end rel err ~8e-4 vs 1.7e-3 for bf16). Both halves run
in DoubleRow, so the whole GEMM runs at 2x the bf16-kernel speed. The
last N_SKIP_LO of the 8 k-super-steps optionally skip the lo correction
(each skipped step adds sqrt(1/8)*2.7e-2 in quadrature to the error and
saves 1/16 of the PE time).

scale (fp32) and bias (fp32) are fused into the PSUM->SBUF eviction:
out = psum * scale + bias, written as bf16 (adds ~1e-3 rounding, halves
store traffic); the host concatenates, transposes and upcasts.

Layout: host ships the input as two fp8 planes inH/inL of shape
[D_IN, TOKENS] (k-major) and the weight shard transposed (wT = W.T
shard, bf16). k is split (ks, i, p) = (super-step, DoubleRow plane,
partition): k = ks*256 + i*128 + p, a natural C-order reshape on both
operands so no host shuffling beyond the transpose.

Perf notes (cost-model profiled):
- 2048 DoubleRow matmuls of [K=256]x[M=128 o]x[N=512 t] at ~107 ns each
  (~218 us PE busy); the bf16 equivalent was ~438 us.
- DMA totals ~151 us (input hi+lo 33.5 MB, weights 4.2 MB, output bf16
  16.7 MB) at the 360 B/ns aggregate ring rate - hidden under the PE.
- Stationary sign tiles are fp8 ([128,2,128] per (ks,o)); each is reused
  for hi+lo x all token chunks, and redundant Tile-emitted LDWEIGHTS are
  deduped.
- Output stores issue on the ACT HWDGE ring so they can't head-of-line
  block the next span's input loads on the SP ring (strict per-ring FIFO).
- The scale chain never touches the in-order PE queue (cross-partition
  sum via DRAM bounce + DVE, broadcast via step-0 DMA), and its small
  DMAs stay off the SP HWDGE FIFO so they can't head-of-line block the
  input loads while waiting on the collective.
- Ramped token spans (512,512,1024,2048x3): early spans use 1 PSUM bank
  per o-group (up to 8 in flight) and a copy-only eviction with the
  scale/bias folded in a second DVE pass, so nothing stalls on the
  AllReduce latency.
"""

import sys

for _p in ("/opt/trn_rl_repo",):
    if _p not in sys.path:
        sys.path.append(_p)

import ml_dtypes
import numpy as np

TOKENS = 8192
D_IN = 2048
D_OUT = 8192
NCORES = 8
OSH = D_OUT // NCORES  # 1024 out features per core
P = 128
KS = D_IN // (2 * P)   # 8 k-super-tiles of 256 (two DoubleRow planes)
TQ = 2048              # resident token span
OT = OSH // P          # 8 o-tiles per core
SPAN_SCHEDULE = (512, 512, 1024, 2048, 2048, 2048)
N_SKIP_LO = 0          # k-super-steps (from the end) without lo correction

_NC_CACHE = {}


def _build_nc(use_collective=True, repeat=1, dedup_ldw=True,
              n_skip_lo=N_SKIP_LO):
    import concourse.mybir as mybir
    import concourse.tile as tile
    from concourse import bacc

    f32 = mybir.dt.float32
    bf16 = mybir.dt.bfloat16
    fp8 = mybir.dt.float8e4
    AF = mybir.ActivationFunctionType
    DR = mybir.MatmulPerfMode.DoubleRow

    nc = bacc.Bacc("TRN2", target_bir_lowering=False, debug=False,
                   num_devices=NCORES)

    inH = nc.dram_tensor("inH", [D_IN, TOKENS], fp8, kind="ExternalInput")
    inL = nc.dram_tensor("inL", [D_IN, TOKENS], fp8, kind="ExternalInput")
    wT = nc.dram_tensor("wT", [D_IN, OSH], bf16, kind="ExternalInput")
    bias2d = nc.dram_tensor("bias2d", [P, OT], f32, kind="ExternalInput")
    outT = nc.dram_tensor("outT", [OSH, TOKENS], bf16, kind="ExternalOutput")
    cc_in = nc.dram_tensor("cc_in", [1, 8], f32)
    cc_out = nc.dram_tensor("cc_out", [1, 8], f32, addr_space="Shared")
    colsum_dram = nc.dram_tensor("colsum_dram", [P], f32)

    # k = ks*256 + i*128 + p (natural C-order reshape)
    inH_r = inH.ap().rearrange("(ks i p) t -> p ks i t", i=2, p=P)
    inL_r = inL.ap().rearrange("(ks i p) t -> p ks i t", i=2, p=P)
    # kk = ks*2 + i: plane-major k-tile index of 128
    wT_r = wT.ap().rearrange("(kk p) o -> p kk o", p=P)
    outT_r = outT.ap().rearrange("(o p) t -> p o t", p=P)

    # W DMA schedule in k-super (256-k) units: small first load so the first
    # stationary tile (and the first matmul) is ready a few us in.
    if KS == 8:
        WSCHED = (1, 1, 2, 2, 2)
    else:
        WSCHED = (KS,)
    NWQ = len(WSCHED)
    WQMAX = max(WSCHED)

    with tile.TileContext(nc) as tc:
        with (
            tc.tile_pool(name="const", bufs=1) as const,
            tc.tile_pool(name="wpool", bufs=1) as wpool,
            tc.tile_pool(name="wstream", bufs=2) as wstream,
            tc.tile_pool(name="small", bufs=1) as small,
            tc.tile_pool(name="inpool", bufs=28) as inpool,
            tc.tile_pool(name="outpool", bufs=2) as outpool,
            tc.tile_pool(name="pmm", bufs=8, space="PSUM") as pmm,
        ):
            bias_sb = const.tile([P, OT], f32)
            nc.gpsimd.dma_start(bias_sb[:], bias2d.ap())

            # PE clock warmup: the HAM gate holds the array at 1.2 GHz until
            # ~3.4us of sustained activity. Burn that window on throwaway
            # matmuls over a zeroed tile while the first weights stream in,
            # so the real matmuls start at 2.4 GHz.
            warm_src = const.tile([P, 256], bf16)
            nc.vector.memset(warm_src[:], 0.0)
            warm_ps = pmm.tile([P, 512], f32, tag="mm", name="warm_ps")
            NWARM = 14
            for wmm in range(NWARM):
                nc.tensor.matmul(warm_ps[0:16, 0:256], warm_src[:, 0:16],
                                 warm_src[:],
                                 start=(wmm == 0), stop=(wmm == NWARM - 1))

            # --- weight shard: sign -> fp8 (DoubleRow layout), |W| partials ---
            # Sign on ACT; |.| row-sums on DVE (reduce with absolute value);
            # no PE involvement anywhere in the scale chain so the in-order
            # PE queue is never blocked on it.
            sT = wpool.tile([P, KS, 2, OSH], fp8)
            absacc = wpool.tile([P, NWQ], f32)
            k0 = 0
            for g, wq in enumerate(WSCHED):
                wt = wstream.tile([P, 2 * WQMAX, OSH], bf16, tag="wt",
                                  name=f"wt{g}")
                nc.sync.dma_start(
                    wt[:, :2 * wq, :], wT_r[:, 2 * k0:2 * (k0 + wq), :]
                )
                for s in range(wq):
                    nc.scalar.activation(sT[:, k0 + s, :, :],
                                         wt[:, 2 * s:2 * s + 2, :], AF.Sign)
                nc.vector.tensor_reduce(absacc[:, g:g + 1], wt[:, :2 * wq, :],
                                        axis=mybir.AxisListType.XY,
                                        op=mybir.AluOpType.add,
                                        apply_absolute_value=True)
                k0 += wq

            # --- global scale via AllReduce of the scalar partial ---
            colsum = small.tile([P, 1], f32)
            nc.vector.reduce_sum(colsum[:], absacc[:], axis=mybir.AxisListType.X)
            # cross-partition gather via a DRAM bounce (partition axis can't
            # fold into an SBUF free axis) + free-axis reduce
            nc.gpsimd.dma_start(colsum_dram.ap(), colsum[:, 0])
            rowt = small.tile([1, P], f32)
            nc.gpsimd.dma_start(rowt[0:1, :], colsum_dram.ap()[None, :])
            part = small.tile([1, 8], f32)
            nc.vector.memset(part[:], 0.0)
            nc.vector.reduce_sum(part[0:1, 0:1], rowt[0:1, :],
                                 axis=mybir.AxisListType.X)
            # keep the scale chain's DMAs off the SP HWDGE ring: tot8 waits
            # on the collective, and the SP ring is FIFO — it would
            # head-of-line block every subsequent input load.
            nc.gpsimd.dma_start(cc_in.ap(), part[:])
            if use_collective:
                nc.gpsimd.collective_compute(
                    "AllReduce",
                    mybir.AluOpType.add,
                    replica_groups=[list(range(NCORES))],
                    ins=[cc_in.ap()],
                    outs=[cc_out.ap()],
                )
                cc_result = cc_out
            else:
                # timing-model variant (TimelineSim can't model collectives):
                # local partial stands in for the global sum
                nc.gpsimd.dma_start(cc_out.ap(), cc_in.ap())
                cc_result = cc_out
            # broadcast the reduced scalar to all 128 partitions straight
            # from DRAM (step-0 source AP)
            scale_raw = small.tile([P, 1], f32)
            with nc.allow_non_contiguous_dma(reason="scale broadcast"):
                nc.gpsimd.dma_start(scale_raw[:, 0:1],
                                    cc_result.ap()[0:1, 0:1].to_broadcast((P, 1)))
            scale_b = small.tile([P, 1], f32)
            nc.scalar.activation(scale_b[:], scale_raw[:], AF.Copy,
                                 scale=1.0 / float(D_OUT * D_IN))

            # --- main GEMM: outT[o, t] = sum_k sT[k, o] * (xhi+xlo)[k, t] ---
            # DoubleRow fp8: each matmul contracts 256 k (2 planes x 128
            # partitions) at 0.5 cycles per output column. hi and lo input
            # planes accumulate into the same PSUM bank; the sign stationary
            # is shared by both (and by all token chunks) per (ks, o).
            spans = []
            t0 = 0
            for tq in SPAN_SCHEDULE:
                spans.append((t0, tq))
                t0 += tq
            assert t0 == TOKENS
            # repeat>1 re-runs the whole GEMM (same outputs rewritten) so a
            # wall-clock slope over R cancels fixed launch/proxy overheads.
            spans = [(q + r * len(spans), t0, tq)
                     for r in range(repeat)
                     for q, (t0, tq) in enumerate(spans)]
            nspans0 = len(SPAN_SCHEDULE)
            for q, t0, tq in spans:
                ncht = tq // 512
                inq = []
                for ks in range(KS):
                    ih = inpool.tile([P, 2, TQ], fp8, tag="in",
                                     name=f"inh_q{q}_k{ks}")
                    nc.sync.dma_start(ih[:, :, :tq], inH_r[:, ks, :, t0:t0 + tq])
                    if ks < KS - n_skip_lo:
                        il = inpool.tile([P, 2, TQ], fp8, tag="in",
                                         name=f"inl_q{q}_k{ks}")
                        nc.sync.dma_start(il[:, :, :tq],
                                          inL_r[:, ks, :, t0:t0 + tq])
                    else:
                        il = None
                    inq.append((ih, il))
                for o in range(OT):
                    psums = [
                        pmm.tile([P, 512], f32, tag="mm", name=f"pp{q}_{o}_{c}")
                        for c in range(ncht)
                    ]
                    for ks in range(KS):
                        lhsT = sT[:, ks, :, o * P:(o + 1) * P]
                        parts = [inq[ks][0]]
                        if inq[ks][1] is not None:
                            parts.append(inq[ks][1])
                        for pi, src in enumerate(parts):
                            is_last = (ks == KS - 1 and pi == len(parts) - 1)
                            for c in range(ncht):
                                nc.tensor.matmul(
                                    psums[c][:], lhsT,
                                    src[:, :, c * 512:(c + 1) * 512],
                                    start=(ks == 0 and pi == 0),
                                    stop=is_last,
                                    perf_mode=DR,
                                )
                    stage = outpool.tile([P, tq], bf16, tag=f"stage{tq}",
                                         bufs=(8 if tq <= 512 else 2),
                                         name=f"st{q}_{o}")
                    if q % nspans0 < 3 and q < nspans0:
                        # early spans: scale may still be in flight (the
                        # AllReduce) — evict with a plain copy so the PSUM
                        # bank frees immediately, fold scale+bias in a
                        # second DVE pass before the store.
                        for c in range(ncht):
                            nc.scalar.activation(
                                stage[:, c * 512:(c + 1) * 512], psums[c][:],
                                AF.Copy)
                        nc.vector.tensor_scalar(
                            stage[:], stage[:],
                            scale_b[:, 0:1], bias_sb[:, o:o + 1],
                            mybir.AluOpType.mult, mybir.AluOpType.add)
                    elif q == len(spans) - 1 and o == OT - 1:
                        # very last tile: store per chunk so the final DMA
                        # isn't serialized behind all four evictions
                        for c in range(ncht):
                            nc.scalar.activation(
                                stage[:, c * 512:(c + 1) * 512], psums[c][:],
                                AF.Identity,
                                bias=bias_sb[:, o:o + 1], scale=scale_b[:, 0:1],
                            )
                            eng = nc.scalar if c % 2 == 0 else nc.sync
                            eng.dma_start(
                                outT_r[:, o, t0 + c * 512:t0 + (c + 1) * 512],
                                stage[:, c * 512:(c + 1) * 512])
                        continue
                    else:
                        for c in range(ncht):
                            nc.scalar.activation(
                                stage[:, c * 512:(c + 1) * 512], psums[c][:],
                                AF.Identity,
                                bias=bias_sb[:, o:o + 1], scale=scale_b[:, 0:1],
                            )
                    nc.scalar.dma_start(outT_r[:, o, t0:t0 + tq],
                                      stage[:])

    if dedup_ldw:
        _dedup_ldweights(nc, mybir)
    nc.compile()
    return nc


def _dedup_ldweights(nc, mybir):
    """Drop consecutive InstLdweights that reload the exact same stationary
    AP with only matmuls in between. Tile emits one weight load per matmul
    even when several matmuls share a stationary; the following
    non-self-loading matmuls keep using the already-loaded array state.
    Only waitless/updateless loads are removed."""
    removed = 0
    for bb in nc.m.functions[0].blocks:
        il = bb.instructions
        kept = []
        prev_sig = None
        for i in il:
            if isinstance(i, mybir.InstLdweights):
                sig = str(i.ins[0])
                if (sig == prev_sig and not i.has_wait()
                        and not i.has_update()):
                    nc.inst_map.pop(i.name, None)
                    removed += 1
                    continue
                prev_sig = sig
            elif isinstance(i, mybir.InstMatmult):
                pass
            elif getattr(i, "engine", None) == mybir.EngineType.PE:
                prev_sig = None
            kept.append(i)
        il[:] = kept


def _get_nc():
    if "nc" not in _NC_CACHE:
        _NC_CACHE["nc"] = _build_nc()
    return _NC_CACHE["nc"]


def _make_in_maps(input, weight, bias):
    inT = np.ascontiguousarray(input.T)
    inT_hi = inT.astype(ml_dtypes.float8_e4m3)
    inT_lo = (inT - inT_hi.astype(np.float32)).astype(ml_dtypes.float8_e4m3)
    wT_full = weight.T  # [D_IN, D_OUT] view
    in_maps = []
    for j in range(NCORES):
        bsh = bias[j * OSH:(j + 1) * OSH]
        in_maps.append({
            "inH": inT_hi,
            "inL": inT_lo,
            "wT": np.ascontiguousarray(
                wT_full[:, j * OSH:(j + 1) * OSH]).astype(ml_dtypes.bfloat16),
            "bias2d": np.ascontiguousarray(
                bsh.reshape(OT, P).T, dtype=np.float32),
        })
    return in_maps


def run(input, weight, bias, trace=False, **spmd_kwargs):
    from concourse.bass_utils import run_bass_kernel_spmd

    nc = _get_nc()
    in_maps = _make_in_maps(np.asarray(input, dtype=np.float32),
                            np.asarray(weight, dtype=np.float32),
                            np.asarray(bias, dtype=np.float32))
    res = run_bass_kernel_spmd(nc, in_maps, core_ids=list(range(NCORES)),
                               trace=trace, **spmd_kwargs)
    outT = np.concatenate([r["outT"] for r in res.results], axis=0)
    out = np.ascontiguousarray(outT.T.astype(np.float32))
    return out, res


def kernel(input, weight, bias):
    out, _ = run(input, weight, bias, trace=False)
    return out


# revision 18
# speedup vs baseline: 2.1814x; 1.2256x over previous
"""BitLinear (BitNet-style) kernel for 8 Trainium2 NeuronCores.

Computes: out = input @ (sign(W) * mean(|W|)).T + bias
  input [8192, 2048] f32, W [8192, 2048] f32, bias [8192] f32 -> out [8192, 8192] f32

Sharding: column-parallel over out_features. Core j owns W rows
[j*1024, (j+1)*1024). Each core computes sign() on its shard (scalar
engine) and a local |W| partial sum (vector engine reduce with absolute
value); partial sums are AllReduce'd across the 8 cores so the scale is
the global abs-mean.

GEMM precision/speed: the PE's fp8 DoubleRow mode packs two k-planes per
matmul (stationary [128,2,M], moving [128,2,N]) and streams at 0.5
cycles per output column - 2x the bf16 column rate with twice the K per
step. sign(W) is exactly representable in fp8e4, and the input is fed as
an exact-ish hi+lo pair: x_hi = fp8(x), x_lo = fp8(x - x_hi), both
multiplied against the same sign stationary into the same PSUM
accumulation, recovering ~11 mantissa bits. The last N_SKIP_LO of the 8
k-super-steps skip the lo correction (each skipped step adds
sqrt(1/8)*2.7e-2 in quadrature; N_SKIP_LO=2 measures 1.37e-2 end-to-end
vs the 2e-2 gate) and save 1/16 of the PE time each.

Weights ship as fp8e4 of (W.T * 2048): sign is preserved (only |w| <
4.8e-7 quantizes to 0 - 119 of 16.7M elements, ~2e-3 quadrature error)
and the |W| partial sums come out scaled by 2048, folded into the final
scale constant. This halves weight DMA vs bf16 and gets the first
stationary ready sooner.

scale (fp32) and bias (fp32) are fused into the PSUM->SBUF eviction:
out = psum * scale + bias, written as bf16 (~1e-3 rounding, halves
store traffic); the host concatenates, transposes and upcasts.

Layout: host ships the input as two fp8 planes inH/inL of shape
[D_IN, TOKENS] (k-major). k is split (ks, i, p) = (super-step, DoubleRow
plane, partition): k = ks*256 + i*128 + p, a natural C-order reshape on
both operands so no host shuffling beyond the transpose.

Perf notes (cost-model profiled):
- 1792 DoubleRow matmuls of [K=256]x[M=128 o]x[N=512 t] at ~107 ns each
  (~191 us PE busy); the bf16 kernel's floor was ~438 us.
- 16 uniform 512-token spans: input arrives in 364 ns quanta so the PE
  is never waiting on a half-loaded 2048-token span; steady-state DMA
  per span (~8 us) is well under PE per span (~12 us).
- Weights load on the ACT HWDGE ring so the SP ring is pure input loads
  from t=0; span 0 runs ks-outer (all 8 PSUM banks open) so the PE
  consumes each sign plane as the ACT engine produces it.
- Per-span staging tile [128, 8, 512] bf16 and ONE batched SWDGE store
  per span: the Pool engine pays the 994 ns SWDGE overhead 16x instead
  of 128x, and stores never sit on the ACT/SP sequencers where they
  could head-of-line block evictions or input loads.
- The scale chain never touches the in-order PE queue (cross-partition
  sum via DRAM bounce + DVE, broadcast via step-0 DMA), and its small
  DMAs stay on the SWDGE queue so they can't head-of-line block input
  loads while waiting on the collective.
- First 3 spans evict with a plain copy and fold scale+bias in a second
  DVE pass, so nothing stalls on the AllReduce latency.
"""

import sys

for _p in ("/opt/trn_rl_repo",):
    if _p not in sys.path:
        sys.path.append(_p)

import ml_dtypes
import numpy as np

TOKENS = 8192
D_IN = 2048
D_OUT = 8192
NCORES = 8
OSH = D_OUT // NCORES  # 1024 out features per core
P = 128
KS = D_IN // (2 * P)   # 8 k-super-tiles of 256 (two DoubleRow planes)
OT = OSH // P          # 8 o-tiles per core
SPAN = 512
NSPAN = TOKENS // SPAN
EARLY = 3              # spans evicted before the scale is known
N_SKIP_LO = 2          # k-super-steps (from the end) without lo correction
W_PRESCALE = 2048.0    # host premultiplier so fp8(W.T) keeps tiny signs

_NC_CACHE = {}


def _build_nc(use_collective=True, repeat=1, dedup_ldw=True,
              n_skip_lo=N_SKIP_LO):
    import concourse.mybir as mybir
    import concourse.tile as tile
    from concourse import bacc

    f32 = mybir.dt.float32
    bf16 = mybir.dt.bfloat16
    fp8 = mybir.dt.float8e4
    AF = mybir.ActivationFunctionType
    DR = mybir.MatmulPerfMode.DoubleRow

    nc = bacc.Bacc("TRN2", target_bir_lowering=False, debug=False,
                   num_devices=NCORES)

    inH = nc.dram_tensor("inH", [D_IN, TOKENS], fp8, kind="ExternalInput")
    inL = nc.dram_tensor("inL", [D_IN, TOKENS], fp8, kind="ExternalInput")
    wT = nc.dram_tensor("wT", [D_IN, OSH], fp8, kind="ExternalInput")
    bias2d = nc.dram_tensor("bias2d", [P, OT], f32, kind="ExternalInput")
    outT = nc.dram_tensor("outT", [OSH, TOKENS], bf16, kind="ExternalOutput")
    cc_in = nc.dram_tensor("cc_in", [1, 8], f32)
    cc_out = nc.dram_tensor("cc_out", [1, 8], f32, addr_space="Shared")
    colsum_dram = nc.dram_tensor("colsum_dram", [P], f32)

    # k = ks*256 + i*128 + p (natural C-order reshape)
    inH_r = inH.ap().rearrange("(ks i p) t -> p ks i t", i=2, p=P)
    inL_r = inL.ap().rearrange("(ks i p) t -> p ks i t", i=2, p=P)
    # kk = ks*2 + i: plane-major k-tile index of 128
    wT_r = wT.ap().rearrange("(kk p) o -> p kk o", p=P)
    outT_r = outT.ap().rearrange("(o p) t -> p o t", p=P)

    # W DMA schedule in k-super (256-k) units: small first loads so the
    # first stationary tiles are ready a couple of us in.
    if KS == 8:
        WSCHED = (1, 1, 2, 2, 2)
    else:
        WSCHED = (KS,)
    NWQ = len(WSCHED)
    WQMAX = max(WSCHED)

    with tile.TileContext(nc) as tc:
        with (
            tc.tile_pool(name="const", bufs=1) as const,
            tc.tile_pool(name="wpool", bufs=1) as wpool,
            tc.tile_pool(name="wstream", bufs=2) as wstream,
            tc.tile_pool(name="small", bufs=1) as small,
            tc.tile_pool(name="inpool", bufs=42) as inpool,
            tc.tile_pool(name="outpool", bufs=3) as outpool,
            tc.tile_pool(name="pmm", bufs=8, space="PSUM") as pmm,
        ):
            bias_sb = const.tile([P, OT], f32)
            nc.gpsimd.dma_start(bias_sb[:], bias2d.ap())

            # PE clock warmup: the HAM gate holds the array at 1.2 GHz until
            # ~3.4us of sustained activity. Burn that window on throwaway
            # matmuls over a zeroed tile while the first weights stream in,
            # so the real matmuls start at 2.4 GHz.
            warm_src = const.tile([P, 256], bf16)
            nc.vector.memset(warm_src[:], 0.0)
            warm_ps = pmm.tile([P, 512], f32, tag="mm", name="warm_ps")
            NWARM = 17
            for wmm in range(NWARM):
                nc.tensor.matmul(warm_ps[0:16, 0:256], warm_src[:, 0:16],
                                 warm_src[:],
                                 start=(wmm == 0), stop=(wmm == NWARM - 1))

            # --- weight shard: sign -> fp8 (DoubleRow layout), |W| partials ---
            # Prologue: weight chunks interleaved with span-0 input loads on
            # the SP ring, so sign planes and span-0 inputs arrive in the
            # order the ks-outer span-0 loop consumes them. All DMA issues
            # precede the signs; each sign only waits on its own chunk's
            # completion semaphore.
            # Sign on ACT; |.| row-sums on DVE; no PE involvement anywhere in
            # the scale chain so the in-order PE queue is never blocked on it.
            sT = wpool.tile([P, KS, 2, OSH], fp8)
            absacc = wpool.tile([P, NWQ], f32)

            def issue_in(q, t0, ks):
                ih = inpool.tile([P, 2, SPAN], fp8, tag="in",
                                 name=f"inh{q}_{ks}")
                nc.sync.dma_start(ih[:], inH_r[:, ks, :, t0:t0 + SPAN])
                il = None
                if ks < KS - n_skip_lo:
                    il = inpool.tile([P, 2, SPAN], fp8, tag="in",
                                     name=f"inl{q}_{ks}")
                    nc.sync.dma_start(il[:], inL_r[:, ks, :, t0:t0 + SPAN])
                return (ih, il)

            wts = []
            in0 = []
            k0 = 0
            for g, wq in enumerate(WSCHED):
                wt = wstream.tile([P, 2 * WQMAX, OSH], fp8, tag="wt",
                                  bufs=NWQ, name=f"wt{g}")
                nc.sync.dma_start(
                    wt[:, :2 * wq, :], wT_r[:, 2 * k0:2 * (k0 + wq), :]
                )
                wts.append((wt, k0, wq))
                for ks in range(k0, k0 + wq):
                    in0.append(issue_in(0, 0, ks))
                k0 += wq
            for g, (wt, k0, wq) in enumerate(wts):
                for s in range(wq):
                    if k0 + s == 0:
                        # first plane in o-halves: the first stationary
                        # (ks0, o0) is ready ~1 us sooner
                        for h in range(2):
                            nc.scalar.activation(
                                sT[:, 0, :, h * 512:(h + 1) * 512],
                                wt[:, 0:2, h * 512:(h + 1) * 512], AF.Sign)
                    else:
                        nc.scalar.activation(sT[:, k0 + s, :, :],
                                             wt[:, 2 * s:2 * s + 2, :],
                                             AF.Sign)
                nc.vector.tensor_reduce(absacc[:, g:g + 1], wt[:, :2 * wq, :],
                                        axis=mybir.AxisListType.XY,
                                        op=mybir.AluOpType.add,
                                        apply_absolute_value=True)

            # --- global scale via AllReduce of the scalar partial ---
            colsum = small.tile([P, 1], f32)
            nc.vector.reduce_sum(colsum[:], absacc[:], axis=mybir.AxisListType.X)
            # cross-partition gather via a DRAM bounce (partition axis can't
            # fold into an SBUF free axis) + free-axis reduce
            nc.gpsimd.dma_start(colsum_dram.ap(), colsum[:, 0])
            rowt = small.tile([1, P], f32)
            nc.gpsimd.dma_start(rowt[0:1, :], colsum_dram.ap()[None, :])
            part = small.tile([1, 8], f32)
            nc.vector.memset(part[:], 0.0)
            nc.vector.reduce_sum(part[0:1, 0:1], rowt[0:1, :],
                                 axis=mybir.AxisListType.X)
            nc.gpsimd.dma_start(cc_in.ap(), part[:])
            if use_collective:
                nc.gpsimd.collective_compute(
                    "AllReduce",
                    mybir.AluOpType.add,
                    replica_groups=[list(range(NCORES))],
                    ins=[cc_in.ap()],
                    outs=[cc_out.ap()],
                )
                cc_result = cc_out
            else:
                # timing-model variant (TimelineSim can't model collectives):
                # local partial stands in for the global sum
                nc.gpsimd.dma_start(cc_out.ap(), cc_in.ap())
                cc_result = cc_out
            # broadcast the reduced scalar to all 128 partitions straight
            # from DRAM (step-0 source AP)
            scale_raw = small.tile([P, 1], f32)
            with nc.allow_non_contiguous_dma(reason="scale broadcast"):
                nc.gpsimd.dma_start(scale_raw[:, 0:1],
                                    cc_result.ap()[0:1, 0:1].to_broadcast((P, 1)))
            scale_b = small.tile([P, 1], f32)
            nc.scalar.activation(scale_b[:], scale_raw[:], AF.Copy,
                                 scale=1.0 / float(D_OUT * D_IN * W_PRESCALE))

            # --- main GEMM: outT[o, t] = sum_k sT[k, o] * (xhi+xlo)[k, t] ---
            # DoubleRow fp8: each matmul contracts 256 k (2 planes x 128
            # partitions) at 0.5 cycles per output column. hi and lo input
            # planes accumulate into the same PSUM bank; the sign stationary
            # is shared by both per (ks, o).
            spans = [(q + r * NSPAN, (q % NSPAN) * SPAN)
                     for r in range(repeat) for q in range(NSPAN)]
            nlo = KS - n_skip_lo

            def mm(ps, o, ks, pi, src, nparts):
                nc.tensor.matmul(
                    ps[:], sT[:, ks, :, o * P:(o + 1) * P], src[:],
                    start=(ks == 0 and pi == 0),
                    stop=(ks == KS - 1 and pi == nparts - 1),
                    perf_mode=DR,
                )

            def evict(stage, ps, o, early):
                if early:
                    # scale may still be in flight (the AllReduce) - plain
                    # copy frees the PSUM bank now; scale+bias folded in a
                    # DVE pass afterwards.
                    nc.scalar.activation(stage[:, o, :], ps[:], AF.Copy)
                    nc.vector.tensor_scalar(
                        stage[:, o, :], stage[:, o, :],
                        scale_b[:, 0:1], bias_sb[:, o:o + 1],
                        mybir.AluOpType.mult, mybir.AluOpType.add)
                else:
                    nc.scalar.activation(
                        stage[:, o, :], ps[:], AF.Identity,
                        bias=bias_sb[:, o:o + 1], scale=scale_b[:, 0:1])

            for q, t0 in spans:
                early = q < EARLY
                if q == 0:
                    inq = in0
                else:
                    inq = [issue_in(q, t0, ks) for ks in range(KS)]
                stage = outpool.tile([P, OT, SPAN], bf16, tag="stage",
                                     name=f"st{q}")
                psums = [pmm.tile([P, SPAN], f32, tag="mm",
                                  name=f"pp{q}_{o}") for o in range(OT)]
                if q < 2:
                    # ks-outer: consume each sign plane / input tile as it's
                    # produced; all 8 PSUM banks accumulate simultaneously.
                    # At the last ks, finish + evict per o so banks free for
                    # the next span as the ACT engine catches up.
                    for ks in range(KS - 1):
                        for o in range(OT):
                            for pi, src in enumerate(
                                    s for s in inq[ks] if s is not None):
                                mm(psums[o], o, ks, pi, src,
                                   2 if ks < nlo else 1)
                    ks = KS - 1
                    for o in range(OT):
                        for pi, src in enumerate(
                                s for s in inq[ks] if s is not None):
                            mm(psums[o], o, ks, pi, src, 2 if ks < nlo else 1)
                        evict(stage, psums[o], o, early)
                else:
                    last = q == spans[-1][0]
                    for o in range(OT):
                        for ks in range(KS):
                            for pi, src in enumerate(
                                    s for s in inq[ks] if s is not None):
                                mm(psums[o], o, ks, pi, src,
                                   2 if ks < nlo else 1)
                        evict(stage, psums[o], o, early)
                        if last:
                            # per-o stores right behind each eviction, on
                            # the SP ring - input loads are done by now, so
                            # SP is idle and the ACT queue keeps evicting:
                            # the drain tail is one small DMA
                            nc.sync.dma_start(outT_r[:, o, t0:t0 + SPAN],
                                              stage[:, o, :])
                    if last:
                        continue
                # batched stores per span half on the SWDGE queue (two
                # ~1.5 us device slices interleave with input loads better
                # than one 3 us one)
                h = OT // 2
                nc.gpsimd.dma_start(outT_r[:, 0:h, t0:t0 + SPAN],
                                    stage[:, 0:h, :])
                nc.gpsimd.dma_start(outT_r[:, h:, t0:t0 + SPAN],
                                    stage[:, h:, :])

    if dedup_ldw:
        _dedup_ldweights(nc, mybir)
    nc.compile()
    return nc


def _dedup_ldweights(nc, mybir):
    """Drop consecutive InstLdweights that reload the exact same stationary
    AP with only matmuls in between. Tile emits one weight load per matmul
    even when several matmuls share a stationary; the following
    non-self-loading matmuls keep using the already-loaded array state.
    Only waitless/updateless loads are removed."""
    removed = 0
    for bb in nc.m.functions[0].blocks:
        il = bb.instructions
        kept = []
        prev_sig = None
        for i in il:
            if isinstance(i, mybir.InstLdweights):
                sig = str(i.ins[0])
                if (sig == prev_sig and not i.has_wait()
                        and not i.has_update()):
                    nc.inst_map.pop(i.name, None)
                    removed += 1
                    continue
                prev_sig = sig
            elif isinstance(i, mybir.InstMatmult):
                pass
            elif getattr(i, "engine", None) == mybir.EngineType.PE:
                prev_sig = None
            kept.append(i)
        il[:] = kept


def _get_nc():
    if "nc" not in _NC_CACHE:
        _NC_CACHE["nc"] = _build_nc()
    return _NC_CACHE["nc"]


def _make_in_maps(input, weight, bias):
    inT = np.ascontiguousarray(input.T)
    inT_hi = inT.astype(ml_dtypes.float8_e4m3)
    inT_lo = (inT - inT_hi.astype(np.float32)).astype(ml_dtypes.float8_e4m3)
    wT_full = weight.T  # [D_IN, D_OUT] view
    in_maps = []
    for j in range(NCORES):
        bsh = bias[j * OSH:(j + 1) * OSH]
        in_maps.append({
            "inH": inT_hi,
            "inL": inT_lo,
            "wT": (np.ascontiguousarray(wT_full[:, j * OSH:(j + 1) * OSH])
                   * np.float32(W_PRESCALE)).astype(ml_dtypes.float8_e4m3),
            "bias2d": np.ascontiguousarray(
                bsh.reshape(OT, P).T, dtype=np.float32),
        })
    return in_maps


def run(input, weight, bias, trace=False, **spmd_kwargs):
    from concourse.bass_utils import run_bass_kernel_spmd

    nc = _get_nc()
    in_maps = _make_in_maps(np.asarray(input, dtype=np.float32),
                            np.asarray(weight, dtype=np.float32),
                            np.asarray(bias, dtype=np.float32))
    res = run_bass_kernel_spmd(nc, in_maps, core_ids=list(range(NCORES)),
                               trace=trace, **spmd_kwargs)
    outT = np.concatenate([r["outT"] for r in res.results], axis=0)
    out = np.ascontiguousarray(outT.T.astype(np.float32))
    return out, res


def kernel(input, weight, bias):
    out, _ = run(input, weight, bias, trace=False)
    return out


# revision 19
# speedup vs baseline: 2.3423x; 1.0738x over previous
"""BitLinear (BitNet-style) kernel for 8 Trainium2 NeuronCores.

Computes: out = input @ (sign(W) * mean(|W|)).T + bias
  input [8192, 2048] f32, W [8192, 2048] f32, bias [8192] f32 -> out [8192, 8192] f32

Sharding: column-parallel over out_features. Core j owns W rows
[j*1024, (j+1)*1024). Each core computes sign() on its shard (scalar
engine) and a local |W| partial sum (vector engine reduce with absolute
value); partial sums are AllReduce'd across the 8 cores so the scale is
the global abs-mean.

GEMM precision/speed: the PE's fp8 DoubleRow mode packs two k-planes per
matmul (stationary [128,2,M], moving [128,2,N]) and streams at 0.5
cycles per output column - 2x the bf16 column rate with twice the K per
step. sign(W) is exactly representable in fp8e4, and the input is fed as
an exact-ish hi+lo pair: x_hi = fp8(x), x_lo = fp8(x - x_hi), both
multiplied against the same sign stationary into the same PSUM
accumulation, recovering ~11 mantissa bits. The last N_SKIP_LO of the 8
k-super-steps skip the lo correction (each skipped step adds
sqrt(1/8)*2.7e-2 in quadrature; N_SKIP_LO=2 measures 1.37e-2 end-to-end
vs the 2e-2 gate) and save 1/16 of the PE time each.

Weights ship as fp8e4 of (W.T * 2048): sign is preserved (only |w| <
4.8e-7 quantizes to 0 - 119 of 16.7M elements, ~2e-3 quadrature error)
and the |W| partial sums come out scaled by 2048, folded into the final
scale constant. This halves weight DMA vs bf16 and gets the first
stationary ready sooner.

scale (fp32) and bias (fp32) are fused into the PSUM->SBUF eviction:
out = psum * scale + bias, written as bf16 (~1e-3 rounding, halves
store traffic); the host concatenates, transposes and upcasts.

Layout: host ships the input as two fp8 planes inH/inL of shape
[D_IN, TOKENS] (k-major). k is split (ks, i, p) = (super-step, DoubleRow
plane, partition): k = ks*256 + i*128 + p, a natural C-order reshape on
both operands so no host shuffling beyond the transpose.

Perf notes (cost-model profiled):
- 1792 DoubleRow matmuls of [K=256]x[M=128 o]x[N=512 t] at ~107 ns each
  (~191 us PE busy); the bf16 kernel's floor was ~438 us.
- 16 uniform 512-token spans: input arrives in 364 ns quanta so the PE
  is never waiting on a half-loaded 2048-token span; steady-state DMA
  per span (~8 us) is well under PE per span (~12 us).
- Weights load on the ACT HWDGE ring so the SP ring is pure input loads
  from t=0; span 0 runs ks-outer (all 8 PSUM banks open) so the PE
  consumes each sign plane as the ACT engine produces it.
- Per-span staging tile [128, 8, 512] bf16 and ONE batched SWDGE store
  per span: the Pool engine pays the 994 ns SWDGE overhead 16x instead
  of 128x, and stores never sit on the ACT/SP sequencers where they
  could head-of-line block evictions or input loads.
- The scale chain never touches the in-order PE queue (cross-partition
  sum via DRAM bounce + DVE, broadcast via step-0 DMA), and its small
  DMAs stay on the SWDGE queue so they can't head-of-line block input
  loads while waiting on the collective.
- First 3 spans evict with a plain copy and fold scale+bias in a second
  DVE pass, so nothing stalls on the AllReduce latency.
"""

import sys

for _p in ("/opt/trn_rl_repo",):
    if _p not in sys.path:
        sys.path.append(_p)

import ml_dtypes
import numpy as np

TOKENS = 8192
D_IN = 2048
D_OUT = 8192
NCORES = 8
OSH = D_OUT // NCORES  # 1024 out features per core
P = 128
KS = D_IN // (2 * P)   # 8 k-super-tiles of 256 (two DoubleRow planes)
OT = OSH // P          # 8 o-tiles per core
SPAN = 512
NSPAN = TOKENS // SPAN
EARLY = 3              # spans evicted before the scale is known
N_SKIP_LO = 3          # k-super-steps (from the end) without lo correction
W_PRESCALE = 2048.0    # host premultiplier so fp8(W.T) keeps tiny signs

_NC_CACHE = {}


def _build_nc(use_collective=True, repeat=1, dedup_ldw=True,
              n_skip_lo=N_SKIP_LO):
    import concourse.mybir as mybir
    import concourse.tile as tile
    from concourse import bacc

    f32 = mybir.dt.float32
    bf16 = mybir.dt.bfloat16
    fp8 = mybir.dt.float8e4
    AF = mybir.ActivationFunctionType
    DR = mybir.MatmulPerfMode.DoubleRow

    nc = bacc.Bacc("TRN2", target_bir_lowering=False, debug=False,
                   num_devices=NCORES)

    inH = nc.dram_tensor("inH", [D_IN, TOKENS], fp8, kind="ExternalInput")
    inL = nc.dram_tensor("inL", [D_IN, TOKENS], fp8, kind="ExternalInput")
    wT = nc.dram_tensor("wT", [D_IN, OSH], fp8, kind="ExternalInput")
    bias2d = nc.dram_tensor("bias2d", [P, OT], f32, kind="ExternalInput")
    outT = nc.dram_tensor("outT", [OSH, TOKENS], bf16, kind="ExternalOutput")
    cc_in = nc.dram_tensor("cc_in", [1, 8], f32)
    cc_out = nc.dram_tensor("cc_out", [1, 8], f32, addr_space="Shared")
    colsum_dram = nc.dram_tensor("colsum_dram", [P], f32)

    # k = ks*256 + i*128 + p (natural C-order reshape)
    inH_r = inH.ap().rearrange("(ks i p) t -> p ks i t", i=2, p=P)
    inL_r = inL.ap().rearrange("(ks i p) t -> p ks i t", i=2, p=P)
    # kk = ks*2 + i: plane-major k-tile index of 128
    wT_r = wT.ap().rearrange("(kk p) o -> p kk o", p=P)
    outT_r = outT.ap().rearrange("(o p) t -> p o t", p=P)

    # W DMA schedule in k-super (256-k) units: small first loads so the
    # first stationary tiles are ready a couple of us in.
    if KS == 8:
        WSCHED = (1, 1, 2, 2, 2)
    else:
        WSCHED = (KS,)
    NWQ = len(WSCHED)
    WQMAX = max(WSCHED)

    with tile.TileContext(nc) as tc:
        with (
            tc.tile_pool(name="const", bufs=1) as const,
            tc.tile_pool(name="wpool", bufs=1) as wpool,
            tc.tile_pool(name="wstream", bufs=2) as wstream,
            tc.tile_pool(name="small", bufs=1) as small,
            tc.tile_pool(name="inpool", bufs=42) as inpool,
            tc.tile_pool(name="outpool", bufs=3) as outpool,
            tc.tile_pool(name="pmm", bufs=8, space="PSUM") as pmm,
        ):
            bias_sb = const.tile([P, OT], f32)
            nc.gpsimd.dma_start(bias_sb[:], bias2d.ap())

            # PE clock warmup: the HAM gate holds the array at 1.2 GHz until
            # ~3.4us of sustained activity. Burn that window on throwaway
            # matmuls over a zeroed tile while the first weights stream in,
            # so the real matmuls start at 2.4 GHz.
            warm_src = const.tile([P, 256], bf16)
            nc.vector.memset(warm_src[:], 0.0)
            warm_ps = pmm.tile([P, 512], f32, tag="mm", name="warm_ps")
            NWARM = 17
            for wmm in range(NWARM):
                nc.tensor.matmul(warm_ps[0:16, 0:256], warm_src[:, 0:16],
                                 warm_src[:],
                                 start=(wmm == 0), stop=(wmm == NWARM - 1))

            # --- weight shard: sign -> fp8 (DoubleRow layout), |W| partials ---
            # Prologue: weight chunks interleaved with span-0 input loads on
            # the SP ring, so sign planes and span-0 inputs arrive in the
            # order the ks-outer span-0 loop consumes them. All DMA issues
            # precede the signs; each sign only waits on its own chunk's
            # completion semaphore.
            # Sign on ACT; |.| row-sums on DVE; no PE involvement anywhere in
            # the scale chain so the in-order PE queue is never blocked on it.
            sT = wpool.tile([P, KS, 2, OSH], fp8)
            absacc = wpool.tile([P, NWQ], f32)

            def issue_in(q, t0, ks):
                ih = inpool.tile([P, 2, SPAN], fp8, tag="in",
                                 name=f"inh{q}_{ks}")
                nc.sync.dma_start(ih[:], inH_r[:, ks, :, t0:t0 + SPAN])
                il = None
                if ks < KS - n_skip_lo:
                    il = inpool.tile([P, 2, SPAN], fp8, tag="in",
                                     name=f"inl{q}_{ks}")
                    nc.sync.dma_start(il[:], inL_r[:, ks, :, t0:t0 + SPAN])
                return (ih, il)

            wts = []
            in0 = []
            k0 = 0
            for g, wq in enumerate(WSCHED):
                wt = wstream.tile([P, 2 * WQMAX, OSH], fp8, tag="wt",
                                  bufs=NWQ, name=f"wt{g}")
                nc.sync.dma_start(
                    wt[:, :2 * wq, :], wT_r[:, 2 * k0:2 * (k0 + wq), :]
                )
                wts.append((wt, k0, wq))
                for ks in range(k0, k0 + wq):
                    in0.append(issue_in(0, 0, ks))
                k0 += wq
            for g, (wt, k0, wq) in enumerate(wts):
                for s in range(wq):
                    if k0 + s == 0:
                        # first plane in o-halves: the first stationary
                        # (ks0, o0) is ready ~1 us sooner
                        for h in range(2):
                            nc.scalar.activation(
                                sT[:, 0, :, h * 512:(h + 1) * 512],
                                wt[:, 0:2, h * 512:(h + 1) * 512], AF.Sign)
                    else:
                        nc.scalar.activation(sT[:, k0 + s, :, :],
                                             wt[:, 2 * s:2 * s + 2, :],
                                             AF.Sign)
                nc.vector.tensor_reduce(absacc[:, g:g + 1], wt[:, :2 * wq, :],
                                        axis=mybir.AxisListType.XY,
                                        op=mybir.AluOpType.add,
                                        apply_absolute_value=True)

            # --- global scale via AllReduce of the scalar partial ---
            colsum = small.tile([P, 1], f32)
            nc.vector.reduce_sum(colsum[:], absacc[:], axis=mybir.AxisListType.X)
            # cross-partition gather via a DRAM bounce (partition axis can't
            # fold into an SBUF free axis) + free-axis reduce
            nc.gpsimd.dma_start(colsum_dram.ap(), colsum[:, 0])
            rowt = small.tile([1, P], f32)
            nc.gpsimd.dma_start(rowt[0:1, :], colsum_dram.ap()[None, :])
            part = small.tile([1, 8], f32)
            nc.vector.memset(part[:], 0.0)
            nc.vector.reduce_sum(part[0:1, 0:1], rowt[0:1, :],
                                 axis=mybir.AxisListType.X)
            nc.gpsimd.dma_start(cc_in.ap(), part[:])
            if use_collective:
                nc.gpsimd.collective_compute(
                    "AllReduce",
                    mybir.AluOpType.add,
                    replica_groups=[list(range(NCORES))],
                    ins=[cc_in.ap()],
                    outs=[cc_out.ap()],
                )
                cc_result = cc_out
            else:
                # timing-model variant (TimelineSim can't model collectives):
                # local partial stands in for the global sum
                nc.gpsimd.dma_start(cc_out.ap(), cc_in.ap())
                cc_result = cc_out
            # broadcast the reduced scalar to all 128 partitions straight
            # from DRAM (step-0 source AP)
            scale_raw = small.tile([P, 1], f32)
            with nc.allow_non_contiguous_dma(reason="scale broadcast"):
                nc.gpsimd.dma_start(scale_raw[:, 0:1],
                                    cc_result.ap()[0:1, 0:1].to_broadcast((P, 1)))
            scale_b = small.tile([P, 1], f32)
            nc.scalar.activation(scale_b[:], scale_raw[:], AF.Copy,
                                 scale=1.0 / float(D_OUT * D_IN * W_PRESCALE))

            # --- main GEMM: outT[o, t] = sum_k sT[k, o] * (xhi+xlo)[k, t] ---
            # DoubleRow fp8: each matmul contracts 256 k (2 planes x 128
            # partitions) at 0.5 cycles per output column. hi and lo input
            # planes accumulate into the same PSUM bank; the sign stationary
            # is shared by both per (ks, o).
            spans = [(q + r * NSPAN, (q % NSPAN) * SPAN)
                     for r in range(repeat) for q in range(NSPAN)]
            nlo = KS - n_skip_lo

            def mm(ps, o, ks, pi, src, nparts):
                nc.tensor.matmul(
                    ps[:], sT[:, ks, :, o * P:(o + 1) * P], src[:],
                    start=(ks == 0 and pi == 0),
                    stop=(ks == KS - 1 and pi == nparts - 1),
                    perf_mode=DR,
                )

            def evict(stage, ps, o, early):
                if early:
                    # scale may still be in flight (the AllReduce) - plain
                    # copy frees the PSUM bank now; scale+bias folded in a
                    # DVE pass afterwards.
                    nc.scalar.activation(stage[:, o, :], ps[:], AF.Copy)
                    nc.vector.tensor_scalar(
                        stage[:, o, :], stage[:, o, :],
                        scale_b[:, 0:1], bias_sb[:, o:o + 1],
                        mybir.AluOpType.mult, mybir.AluOpType.add)
                else:
                    nc.scalar.activation(
                        stage[:, o, :], ps[:], AF.Identity,
                        bias=bias_sb[:, o:o + 1], scale=scale_b[:, 0:1])

            for q, t0 in spans:
                early = q < EARLY
                if q == 0:
                    inq = in0
                else:
                    inq = [issue_in(q, t0, ks) for ks in range(KS)]
                stage = outpool.tile([P, OT, SPAN], bf16, tag="stage",
                                     name=f"st{q}")
                psums = [pmm.tile([P, SPAN], f32, tag="mm",
                                  name=f"pp{q}_{o}") for o in range(OT)]
                if q < 2:
                    # ks-outer: consume each sign plane / input tile as it's
                    # produced; all 8 PSUM banks accumulate simultaneously.
                    # At the last ks, finish + evict per o so banks free for
                    # the next span as the ACT engine catches up.
                    for ks in range(KS - 1):
                        for o in range(OT):
                            for pi, src in enumerate(
                                    s for s in inq[ks] if s is not None):
                                mm(psums[o], o, ks, pi, src,
                                   2 if ks < nlo else 1)
                    ks = KS - 1
                    for o in range(OT):
                        for pi, src in enumerate(
                                s for s in inq[ks] if s is not None):
                            mm(psums[o], o, ks, pi, src, 2 if ks < nlo else 1)
                        evict(stage, psums[o], o, early)
                else:
                    last = q == spans[-1][0]
                    for o in range(OT):
                        for ks in range(KS):
                            for pi, src in enumerate(
                                    s for s in inq[ks] if s is not None):
                                mm(psums[o], o, ks, pi, src,
                                   2 if ks < nlo else 1)
                        evict(stage, psums[o], o, early)
                        if last:
                            # per-o stores right behind each eviction, on
                            # the SP ring - input loads are done by now, so
                            # SP is idle and the ACT queue keeps evicting:
                            # the drain tail is one small DMA
                            nc.sync.dma_start(outT_r[:, o, t0:t0 + SPAN],
                                              stage[:, o, :])
                    if last:
                        continue
                # batched stores per span half on the SWDGE queue (two
                # ~1.5 us device slices interleave with input loads better
                # than one 3 us one)
                h = OT // 2
                nc.gpsimd.dma_start(outT_r[:, 0:h, t0:t0 + SPAN],
                                    stage[:, 0:h, :])
                nc.gpsimd.dma_start(outT_r[:, h:, t0:t0 + SPAN],
                                    stage[:, h:, :])

    if dedup_ldw:
        _dedup_ldweights(nc, mybir)
    nc.compile()
    return nc


def _dedup_ldweights(nc, mybir):
    """Drop consecutive InstLdweights that reload the exact same stationary
    AP with only matmuls in between. Tile emits one weight load per matmul
    even when several matmuls share a stationary; the following
    non-self-loading matmuls keep using the already-loaded array state.
    Only waitless/updateless loads are removed."""
    removed = 0
    for bb in nc.m.functions[0].blocks:
        il = bb.instructions
        kept = []
        prev_sig = None
        for i in il:
            if isinstance(i, mybir.InstLdweights):
                sig = str(i.ins[0])
                if (sig == prev_sig and not i.has_wait()
                        and not i.has_update()):
                    nc.inst_map.pop(i.name, None)
                    removed += 1
                    continue
                prev_sig = sig
            elif isinstance(i, mybir.InstMatmult):
                pass
            elif getattr(i, "engine", None) == mybir.EngineType.PE:
                prev_sig = None
            kept.append(i)
        il[:] = kept


def _get_nc():
    if "nc" not in _NC_CACHE:
        _NC_CACHE["nc"] = _build_nc()
    return _NC_CACHE["nc"]


def _make_in_maps(input, weight, bias):
    inT = np.ascontiguousarray(input.T)
    inT_hi = inT.astype(ml_dtypes.float8_e4m3)
    inT_lo = (inT - inT_hi.astype(np.float32)).astype(ml_dtypes.float8_e4m3)
    wT_full = weight.T  # [D_IN, D_OUT] view
    in_maps = []
    for j in range(NCORES):
        bsh = bias[j * OSH:(j + 1) * OSH]
        in_maps.append({
            "inH": inT_hi,
            "inL": inT_lo,
            "wT": (np.ascontiguousarray(wT_full[:, j * OSH:(j + 1) * OSH])
                   * np.float32(W_PRESCALE)).astype(ml_dtypes.float8_e4m3),
            "bias2d": np.ascontiguousarray(
                bsh.reshape(OT, P).T, dtype=np.float32),
        })
    return in_maps


def run(input, weight, bias, trace=False, **spmd_kwargs):
    from concourse.bass_utils import run_bass_kernel_spmd

    nc = _get_nc()
    in_maps = _make_in_maps(np.asarray(input, dtype=np.float32),
                            np.asarray(weight, dtype=np.float32),
                            np.asarray(bias, dtype=np.float32))
    res = run_bass_kernel_spmd(nc, in_maps, core_ids=list(range(NCORES)),
                               trace=trace, **spmd_kwargs)
    outT = np.concatenate([r["outT"] for r in res.results], axis=0)
    out = np.ascontiguousarray(outT.T.astype(np.float32))
    return out, res


def kernel(input, weight, bias):
    out, _ = run(input, weight, bias, trace=False)
    return out
